# revision 36
# baseline (speedup 1.0000x reference)
"""MoE (14 routed experts top-2 + 2 shared) on 8 TRN2 NeuronCores.

Strategy: expert parallelism. Each core runs an identical Bass/Tile program
with 3 "expert slots" (2 routed, capacity 384 tokens each; 1 shared chunk,
512 tokens). Token->slot assignment is computed on host, passed as per-core
index/gate inputs. The device does: indirect-DMA token gather, RMSNorm
(fused into PE transpose via diag-scale), two bf16 matmuls (f32 accum),
SiLU, gate scaling, and writes dispatch-order outputs. Host combines with
np.add.at.

Core c: routed slots hold experts (2c, 2c+1) for c<6; cores 6,7 hold
expert 12/13 split across both slots. Shared: core c handles shared expert
c//4 on token range [(c%4)*512, +512).
"""

import os
import numpy as np

KVAR = os.environ.get("KVAR", "full")  # bisection knob: full|nogather|dmaonly|noffn

N_CORES = 8
P = 128
D = 1024
H = 2048
N_TOK = 2048
E_ROUTED = 14
E_SHARED = 2
TOP_K = 2
EPS = 1e-9
RMS_EPS = 1.1920929e-07
CAP = 384  # routed slot capacity (tokens), multiple of 128
SH = 512  # shared chunk size per core
SLOT_TILES = (CAP // P, CAP // P, SH // P)  # tiles per slot
N_COLS = sum(SLOT_TILES)  # id/gate columns
SLOT_ROW0 = (0, CAP, 2 * CAP)  # disp_out row offsets
OUT_ROWS = 2 * CAP + SH

_cache = {}


# ---------------------------------------------------------------------------
# Bass program (identical for all 8 cores)
# ---------------------------------------------------------------------------


def _apply_tile_patch():
    """walrus CoreV2/V3 rejects >1 sem wait on CTRL ops; split the Tile
    tail-drain waits across single-wait SP NOPs."""
    import concourse.tile as tile
    import concourse.mybir as mybir
    from concourse.vector_clock import ScopedClock

    def _drain_and_barrier(self, tick_clock, wait_clock):
        probe = self.nc.sync.nop()
        if probe.ins.sync_info is None:
            probe.ins.sync_info = mybir.SyncInfo(on_wait=[], on_update=[])
        wait_clock.add_sem_waits(
            probe.ins, ScopedClock({None: tick_clock.global_clock})
        )
        waits = list(probe.ins.sync_info.on_wait or [])
        probe.ins.sync_info.on_wait = waits[:1]
        for w in waits[1:]:
            n = self.nc.sync.nop()
            if n.ins.sync_info is None:
                n.ins.sync_info = mybir.SyncInfo(on_wait=[], on_update=[])
            n.ins.sync_info.on_wait = [w]
        self.nc.sync.drain()
        self.nc.all_engine_barrier()
        assert self.sems is not None
        popped = self.nc._tile_sem_poison_stack.pop()
        assert popped is self._sem_poison
        self.nc.clear_and_free_semaphores(list(self.sems.allocated().values()))
        self.nc.all_engine_barrier()

    tile.TileContext._drain_and_barrier = _drain_and_barrier


def _split_multi_waits(nc):
    """This walrus build accepts at most one sem-wait per instruction.
    Move extra waits onto single-wait NOPs inserted just before, on the
    same engine (engines execute their stream in order)."""
    import concourse.mybir as mybir

    uid = 0
    for f in nc.m.functions:
        for bb in f.blocks:
            il = bb.instructions
            i = 0
            while i < len(il):
                ins = il[i]
                si = ins.sync_info
                if (
                    si is not None
                    and si.on_wait
                    and len(si.on_wait) > 1
                    and type(ins).__name__ != "InstEventSemaphore"
                ):
                    waits = list(si.on_wait)
                    si.on_wait = waits[-1:]
                    for w in waits[:-1]:
                        n = mybir.InstNoOp(
                            name=f"I-wsplit-{uid}", ins=[], outs=[]
                        )
                        uid += 1
                        n.engine = ins.engine
                        n.sync_info = mybir.SyncInfo(
                            on_wait=[w], on_update=[]
                        )
                        il.insert(i, n)
                        i += 1
                i += 1


def _build_program(for_sim=False):
    import concourse.bass as bass
    import concourse.bacc as bacc
    import concourse.mybir as mybir
    import concourse.tile as tile
    from concourse.masks import make_identity

    _apply_tile_patch()

    f32 = mybir.dt.float32
    bf16 = mybir.dt.bfloat16
    i32 = mybir.dt.int32
    i16 = mybir.dt.int16
    AF = mybir.ActivationFunctionType
    from concourse.tile import add_dep_helper

    nc = bacc.Bacc("TRN2", target_bir_lowering=False, debug=False)

    xb16 = nc.dram_tensor("xb16", [N_TOK, D], bf16, kind="ExternalInput")
    # per-slot weights, SBUF-ready layout:
    # w1l[s][r, (h*8+d)*128 + c] = W1'[s][d*128+r, h*128+c]   (lhsT tiles)
    # w2l[s][r, (o*16+h)*512 + c] = W2'[s][h*128+r, o*512+c]  (rhs tiles)
    w1l = nc.dram_tensor("w1l", [3, P, 16 * 8 * P], bf16, kind="ExternalInput")
    w2l = nc.dram_tensor("w2l", [3, P, 2 * 16 * 512], bf16, kind="ExternalInput")
    # dma_gather index format: idx i of slot s at [16*blk + i%16, icol0(s) + i//16]
    idx16 = nc.dram_tensor(
        "idx16", [P, sum(t * P // 16 for t in SLOT_TILES)], i16,
        kind="ExternalInput",
    )
    gates_t = nc.dram_tensor("gates_t", [P, N_COLS], f32, kind="ExternalInput")
    disp = nc.dram_tensor("disp", [OUT_ROWS, D], f32, kind="ExternalOutput")

    import contextlib

    loop_n = int(os.environ.get("KLOOP", "0"))
    with tile.TileContext(nc) as tc:
        with (
            tc.tile_pool(name="const", bufs=1) as cpool,
            tc.tile_pool(name="w2", bufs=2) as w2pool,
            tc.tile_pool(name="w1", bufs=4) as w1pool,
            tc.tile_pool(name="xs", bufs=2) as xpool,
            tc.tile_pool(name="act", bufs=2) as apool,
            tc.tile_pool(name="ps", bufs=2, space="PSUM") as pspool,
            tc.tile_pool(name="rt", bufs=2) as rpool,
            tc.For_i(0, loop_n, 1) if loop_n else contextlib.nullcontext(),
        ):
            eps_one = cpool.tile([1, 1], f32)
            nc.vector.memset(eps_one[:], RMS_EPS)
            ones_col = cpool.tile([P, 1], f32)
            nc.vector.memset(ones_col[:], 1.0)
            ones_row = cpool.tile([1, P], f32)
            nc.vector.memset(ones_row[:], 1.0)
            gates_all = cpool.tile([P, N_COLS], f32)
            nc.sync.dma_start(out=gates_all[:], in_=gates_t[:])
            idx_all = cpool.tile([P, sum(t * P // 16 for t in SLOT_TILES)], i16)
            nc.sync.dma_start(out=idx_all[:], in_=idx16[:])

            # ---------------- FFN slot pipeline ----------------
            def ffn_slot(slot, idxs_ap, nvalid, gate_col):
                ntile = SLOT_TILES[slot]
                size = ntile * P
                row0 = SLOT_ROW0[slot]

                w2sb = w2pool.tile([P, 2 * 16 * 512], bf16, tag="w2", name=f"w2_{slot}")
                nc.sync.dma_start(out=w2sb[:], in_=w2l[slot])

                xg = xpool.tile([P, 8, size], bf16, tag="xg", name=f"xg_{slot}")
                nc.gpsimd.dma_gather(
                    out_ap=xg[:, :, :size],
                    in_ap=xb16[:],
                    idxs_ap=idxs_ap,
                    num_idxs=size,
                    num_idxs_reg=nvalid,
                    elem_size=D,
                    transpose=True,
                )

                sq = xpool.tile([P, 8, size], f32, tag="sq", name=f"sq_{slot}")
                nc.vector.tensor_mul(sq[:], xg[:], xg[:])
                msp = pspool.tile([1, 512], f32, tag="msp", name=f"msp_{slot}", bufs=1)
                for d in range(8):
                    nc.tensor.matmul(
                        out=msp[:1, :size],
                        lhsT=ones_col[:],
                        rhs=sq[:, d, :size],
                        start=(d == 0),
                        stop=(d == 7),
                    )
                srow = xpool.tile([1, 512], f32, tag="srow", name=f"srow_{slot}")
                nc.scalar.activation(
                    out=srow[:1, :size],
                    in_=msp[:1, :size],
                    func=AF.Sqrt,
                    scale=1.0 / D,
                    bias=eps_one[:1, :1],
                )
                nc.vector.reciprocal(out=srow[:1, :size], in_=srow[:1, :size])
                sbc = pspool.tile([P, 512], f32, tag="sbc", name=f"sbc_{slot}", bufs=1)
                nc.tensor.matmul(
                    out=sbc[:, :size],
                    lhsT=ones_row[:1, :],
                    rhs=srow[:1, :size],
                    start=True,
                    stop=True,
                )
                xn = xpool.tile([P, 8, size], bf16, tag="xn", name=f"xn_{slot}")
                for d in range(8):
                    nc.vector.tensor_mul(
                        xn[:, d, :size], xg[:, d, :size], sbc[:, :size]
                    )

                aT = [
                    apool.tile([P, 512], bf16, tag=f"aT{h}", name=f"aT{h}_{slot}")
                    for h in range(16)
                ]
                for h in range(16):
                    w1sb = w1pool.tile([P, 8 * P], bf16, tag="w1h", name=f"w1h{h}_{slot}", bufs=6)
                    nc.sync.dma_start(
                        out=w1sb[:],
                        in_=w1l[slot, :, h * 8 * P : (h + 1) * 8 * P],
                    )
                    ph = pspool.tile([P, 512], f32, tag="mm1", name=f"mm1_{h}_{slot}", bufs=3)
                    for d in range(8):
                        nc.tensor.matmul(
                            out=ph[:, :size],
                            lhsT=w1sb[:, d * P : (d + 1) * P],
                            rhs=xn[:, d, :size],
                            start=(d == 0),
                            stop=(d == 7),
                        )
                    nc.scalar.activation(
                        out=aT[h][:, :size], in_=ph[:, :size], func=AF.Silu
                    )

                for j in range(ntile):
                    for o in range(2):
                        py = pspool.tile([P, 512], f32, tag="mm2", name=f"mm2_{j}_{o}_{slot}", bufs=3)
                        for h in range(16):
                            nc.tensor.matmul(
                                out=py[:],
                                lhsT=aT[h][:, j * P : (j + 1) * P],
                                rhs=w2sb[:, (o * 16 + h) * 512 : (o * 16 + h + 1) * 512],
                                start=(h == 0),
                                stop=(h == 15),
                            )
                        yo = xpool.tile([P, 512], f32, tag="yo", name=f"yo_{j}_{o}_{slot}")
                        nc.vector.tensor_scalar_mul(yo[:], py[:], gate_col(j))
                        nc.sync.dma_start(
                            out=disp[
                                row0 + j * P : row0 + (j + 1) * P,
                                o * 512 : (o + 1) * 512,
                            ],
                            in_=yo[:],
                        )

            for slot in range(3):
                icol0 = sum(t * P // 16 for t in SLOT_TILES[:slot])
                col0 = sum(SLOT_TILES[:slot])
                size = SLOT_TILES[slot] * P
                ffn_slot(
                    slot,
                    idx_all[:, icol0 : icol0 + size // 16],
                    size,
                    lambda j, c0=col0: gates_all[:, c0 + j : c0 + j + 1],
                )
    nc.finalize()
    if not for_sim:
        _split_multi_waits(nc)
    return nc


# ---------------------------------------------------------------------------
# PJRT runner with cached executable
# ---------------------------------------------------------------------------


def _get_runner():
    if "runner" in _cache:
        return _cache["runner"]

    import jax
    import jax.numpy as jnp
    from jax.sharding import Mesh, PartitionSpec
    from jax.experimental.shard_map import shard_map
    import concourse.mybir as mybir
    from concourse import bass2jax

    bass2jax.install_neuronx_cc_hook()
    nc = _build_program()

    part_name = nc.partition_id_tensor.name if nc.partition_id_tensor else None
    in_names, out_names, out_avals = [], [], []
    for alloc in nc.m.functions[0].allocations:
        if not isinstance(alloc, mybir.MemoryLocationSet):
            continue
        name = alloc.memorylocations[0].name
        if alloc.kind == "ExternalInput":
            if name != part_name:
                in_names.append(name)
        elif alloc.kind == "ExternalOutput":
            out_names.append(name)
            out_avals.append(
                jax.core.ShapedArray(
                    tuple(alloc.tensor_shape), mybir.dt.np(alloc.dtype)
                )
            )
    n_params = len(in_names)
    all_names = in_names + out_names
    if part_name is not None:
        all_names = all_names + [part_name]

    def _body(*args):
        operands = list(args)
        if part_name is not None:
            operands.append(bass2jax.partition_id_tensor())
        outs = bass2jax._bass_exec_p.bind(
            *operands,
            out_avals=tuple(out_avals),
            in_names=tuple(all_names),
            out_names=tuple(out_names),
            lowering_input_output_aliases=(),
            sim_require_finite=True,
            sim_require_nnan=True,
            nc=nc,
        )
        return tuple(outs)

    devices = jax.devices()[:N_CORES]
    mesh = Mesh(np.asarray(devices), ("core",))
    n_args = n_params + len(out_names)
    sharded = jax.jit(
        shard_map(
            _body,
            mesh=mesh,
            in_specs=(PartitionSpec("core"),) * n_args,
            out_specs=(PartitionSpec("core"),) * len(out_names),
            check_rep=False,
        ),
        keep_unused=True,
    )
    from jax.sharding import NamedSharding

    shard = NamedSharding(mesh, PartitionSpec("core"))
    zeros_dev = [
        jax.device_put(
            np.zeros((N_CORES * a.shape[0], *a.shape[1:]), a.dtype), shard
        )
        for a in out_avals
    ]

    def run(in_maps, time_iters=0):
        import time as _time

        concat_in = [
            np.concatenate([np.asarray(m[k]) for m in in_maps], axis=0)
            for k in in_names
        ]
        args_dev = [jax.device_put(a, shard) for a in concat_in]
        for a in args_dev:
            a.block_until_ready()
        out = sharded(*args_dev, *zeros_dev)
        jax.block_until_ready(out)
        if time_iters:
            times = []
            for _ in range(time_iters):
                t0 = _time.perf_counter()
                jax.block_until_ready(sharded(*args_dev, *zeros_dev))
                times.append(_time.perf_counter() - t0)
            _cache["exec_times"] = times
        res = [
            {
                k: np.asarray(out[i]).reshape(N_CORES, *out_avals[i].shape)[c]
                for i, k in enumerate(out_names)
            }
            for c in range(N_CORES)
        ]
        return res

    _cache["runner"] = run
    return run


# ---------------------------------------------------------------------------
# Host-side routing + sharding + combine
# ---------------------------------------------------------------------------


def _route(xf, router_w):
    logits = xf.astype(np.float32) @ router_w.astype(np.float32)
    m = logits.max(axis=-1, keepdims=True)
    e = np.exp(logits - m)
    gates = e / e.sum(axis=-1, keepdims=True)  # [N, E]
    order = np.argsort(-gates, axis=-1, kind="stable")
    top_i = order[:, :TOP_K]
    top_v = np.take_along_axis(gates, top_i, axis=-1)
    w = top_v / (top_v.sum(axis=-1, keepdims=True) + EPS)
    return top_i, w


def _slot_lists(top_i, w):
    """-> per expert: (token_ids, gate_weights), ascending token order."""
    out = []
    for e in range(E_ROUTED):
        mask = top_i == e
        toks = np.where(mask.any(axis=-1))[0]
        kk = np.argmax(mask[toks], axis=-1)
        out.append((toks.astype(np.int64), w[toks, kk].astype(np.float32)))
    return out


def _tile_w1(w1e, rms_we):
    """[D, H] (+ rms fold) -> [128, 16*8*128] bf16-ready f32."""
    wf = w1e * rms_we[:, None]  # fold rmsnorm weight
    t = wf.reshape(8, P, 16, P)  # [d, r, h, c]
    return np.ascontiguousarray(t.transpose(1, 2, 0, 3).reshape(P, 16 * 8 * P))


def _tile_w2(w2e):
    """[H, D] -> [128, 2*16*512]."""
    t = w2e.reshape(16, P, 2, 512)  # [h, r, o, c]
    return np.ascontiguousarray(t.transpose(1, 2, 0, 3).reshape(P, 2 * 16 * 512))


def kernel(x, router_w, rms_w, w1, w2, rms_w_s, w1_s, w2_s):
    import ml_dtypes

    bf16 = ml_dtypes.bfloat16
    x = np.asarray(x, dtype=np.float32)
    B, T, _ = x.shape
    xf = np.ascontiguousarray(x.reshape(N_TOK, D))
    router_w = np.asarray(router_w, np.float32)
    rms_w = np.asarray(rms_w, np.float32)
    w1 = np.asarray(w1, np.float32)
    w2 = np.asarray(w2, np.float32)
    rms_w_s = np.asarray(rms_w_s, np.float32)
    w1_s = np.asarray(w1_s, np.float32)
    w2_s = np.asarray(w2_s, np.float32)

    xf16 = xf.astype(bf16)
    top_i, wts = _route(xf, router_w)
    per_exp = _slot_lists(top_i, wts)

    # expert -> (core, slot) assignment; cores 6/7 split one expert
    slot_exp = np.zeros((N_CORES, 2), np.int64)
    slot_ids = [[None, None] for _ in range(N_CORES)]
    slot_gates = [[None, None] for _ in range(N_CORES)]
    for c in range(6):
        for s in range(2):
            e = 2 * c + s
            slot_exp[c, s] = e
            slot_ids[c][s], slot_gates[c][s] = per_exp[e]
    for c, e in ((6, 12), (7, 13)):
        toks, gs = per_exp[e]
        h = (len(toks) + 1) // 2
        slot_exp[c, 0] = slot_exp[c, 1] = e
        slot_ids[c][0], slot_gates[c][0] = toks[:h], gs[:h]
        slot_ids[c][1], slot_gates[c][1] = toks[h:], gs[h:]

    counts = [len(per_exp[e][0]) for e in range(E_ROUTED)]
    if max(counts) > CAP:  # pathological routing; this build cannot hold it
        raise RuntimeError(f"expert overflow: {counts} > {CAP}")

    w1t_cache, w2t_cache = {}, {}
    in_maps = []
    idx_cols_total = sum(t * P // 16 for t in SLOT_TILES)
    for c in range(N_CORES):
        ids_cols = np.zeros((N_COLS, P), np.int32)
        gate_cols = np.zeros((N_COLS, P), np.float32)
        w1stack = np.empty((3, P, 16 * 8 * P), np.float32)
        w2stack = np.empty((3, P, 2 * 16 * 512), np.float32)
        for s in range(2):
            e = slot_exp[c, s]
            if e not in w1t_cache:
                w1t_cache[e] = _tile_w1(w1[e], rms_w[e])
                w2t_cache[e] = _tile_w2(w2[e])
            w1stack[s] = w1t_cache[e]
            w2stack[s] = w2t_cache[e]
            ids, gs = slot_ids[c][s], slot_gates[c][s]
            col0 = s * (CAP // P)
            ids_cols.reshape(-1)[col0 * P : col0 * P + len(ids)] = ids
            gate_cols.reshape(-1)[col0 * P : col0 * P + len(ids)] = gs
        sid = c // 4
        t0 = (c % 4) * SH
        w1stack[2] = _tile_w1(w1_s[sid], rms_w_s[sid])
        w2stack[2] = _tile_w2(w2_s[sid])
        ids_cols[6:] = np.arange(t0, t0 + SH, dtype=np.int32).reshape(SH // P, P)
        gate_cols[6:] = 1.0
        idx16 = np.zeros((P, idx_cols_total), np.int16)
        for s in range(3):
            nidx = SLOT_TILES[s] * P
            icol0 = sum(t * P // 16 for t in SLOT_TILES[:s])
            flat = ids_cols.reshape(-1)[
                sum(SLOT_TILES[:s]) * P : sum(SLOT_TILES[: s + 1]) * P
            ].astype(np.int16)
            blk = flat.reshape(nidx // 16, 16).T
            idx16[:, icol0 : icol0 + nidx // 16] = np.tile(blk, (8, 1))
        in_maps.append(
            {
                "xb16": xf16,
                "w1l": w1stack.astype(bf16),
                "w2l": w2stack.astype(bf16),
                "idx16": idx16,
                "gates_t": np.ascontiguousarray(gate_cols.T),
            }
        )

    run = _get_runner()
    res = run(in_maps, time_iters=int(os.environ.get("KTIME", "10")))

    acc = np.zeros((N_TOK, D), np.float32)
    for c in range(N_CORES):
        disp = res[c]["disp"]
        for s in range(2):
            ids = slot_ids[c][s]
            if len(ids):
                r0 = SLOT_ROW0[s]
                np.add.at(acc, ids, disp[r0 : r0 + len(ids)])
        t0 = (c % 4) * SH
        acc[t0 : t0 + SH] += disp[SLOT_ROW0[2] : SLOT_ROW0[2] + SH]
    return acc.reshape(B, T, D).astype(np.float32)
